# revision 21
# baseline (speedup 1.0000x reference)
"""Self-contained Trainium2 Bass kernel for nn_AttentionBlock
(B=2, N=2048, D=512, H=8, MLP 2x).

kernel(**inputs) takes the FULL unsharded inputs (as produced by
setup_inputs) and returns the FULL (2, 2048, 512) output.

Sharding: 2-way data-parallel over batch x 4-way parallel over
query-token slices (8 cores, no collectives).  Each core computes K/V
for its whole batch and attention + MLP for its 512-token slice.

v5: three-engine softmax + fp8 DoubleRow scores.
- exp is split across ACT (exact, fp8 out) and DVE/Pool, which emit
  e4m3 *bits* directly with one tensor_scalar (Schraudolph:
  i8 = psc*A8*rstd_j + B8 into a uint8 view of st8; ~7% max per-weight
  error vs ~6% for exact-exp->fp8, so nearly free).
- scores run as fp8 DoubleRow ([64,2,128] stationary K8 with a zeroed
  second slot, stride-0 pair dim on the moving Q8): 107ns per [128,512]
  psc vs 213 bf16.
- LN1 mean is pre-subtracted once (the ones-DR matmul leaves the mean
  replicated across PSUM partitions, so the subtract needs no
  broadcast); stats run in column space; sd_row = var*rstd (no ACT row
  sqrt), so ACT does only 4 tiny col sqrts + exp + gelu, with act
  tables pinned to avoid mid-stream reloads.
"""

from contextlib import ExitStack

import numpy as np

import concourse.bass as bass
import concourse.mybir as mybir
import concourse.tile as tile

_WSPLIT_UID = [0]


def _finalize(nc, max_waits=1):
    """Split multi-sem-wait instructions onto single-wait NoOp carriers
    (the walrus build in this container accepts one wait per instruction)."""
    for f in nc.m.functions:
        for bb in f.blocks:
            insts = bb.instructions
            out = []
            changed = False
            for inst in insts:
                si = inst.sync_info
                waits = list(si.on_wait) if (si and si.on_wait) else []
                if len(waits) > max_waits:
                    changed = True
                    for w in waits[:-max_waits]:
                        _WSPLIT_UID[0] += 1
                        nop = mybir.InstNoOp(
                            name=f"I-wsplit-{_WSPLIT_UID[0]}",
                            ins=[], outs=[], engine=inst.engine,
                        )
                        nop.sync_info = mybir.SyncInfo(on_wait=[w],
                                                       on_update=[])
                        out.append(nop)
                    si.on_wait = waits[-max_waits:]
                out.append(inst)
            if changed:
                bb.instructions = out
    return nc


BF16 = mybir.dt.bfloat16
F32 = mybir.dt.float32
F32R = mybir.dt.float32r
FP8 = mybir.dt.float8e4
U8 = mybir.dt.uint8
AF = mybir.ActivationFunctionType
OP = mybir.AluOpType
DR = mybir.MatmulPerfMode.DoubleRow

P = 128
B, N, D, H = 2, 2048, 512, 8
HD = D // H          # 64
TC = 512             # tokens per core
DM = 2 * D           # 1024 mlp hidden
KC = D // P          # 4 chunks of the 512 feature dim
NT = N // 512        # 4 column tiles of 512 over the 2048 kv tokens
JC = N // P          # 16 token chunks of 128 over kv tokens
MC1 = DM // P        # 8 chunks of mlp hidden
EPS = 1e-5
S = 8.0              # fp8 weight pre-scale
SQH = float(np.sqrt(HD))

A8 = 8.0 / float(np.log(2.0))   # e4m3 octave resolution / ln2
B8 = 56.0 - 0.38                # 7*8 exponent bias + minimax C (hw RTN)


def _mk_sched():
    """128-unit exp schedule (index = (m*16+jc)*2+r)."""
    out = []

    def fill(counts, n):
        acc = {k: 0.0 for k in counts}
        tot = sum(counts.values())
        for _ in range(n):
            for k in acc:
                acc[k] += counts[k] / tot
            k = max(acc, key=lambda q: acc[q])
            acc[k] -= 1.0
            out.append(k)

    fill({"A": 12, "D": 9, "P": 11}, 32)      # wave 0
    for _ in range(2):
        fill({"A": 12, "D": 9, "P": 11}, 32)  # waves 1-2
    fill({"A": 5, "D": 7, "P": 4}, 16)        # wave 3, r=0
    fill({"A": 9, "D": 0, "P": 7}, 16)        # wave 3, r=1: ACT-heavy end
    return "".join(out)


EXP_SCHED = _mk_sched()


def mid_bcast(ap, n, pos=1):
    """Insert a stride-0 broadcast dim at position `pos`."""
    lst = [list(x) for x in ap.ap]
    return bass.AP(tensor=ap.tensor, offset=ap.offset,
                   ap=lst[:pos] + [[0, n]] + lst[pos:])


def build_nc(do_finalize=True):
    nc = bass.Bass()
    x8d = nc.dram_tensor("x8d", [D, N], FP8, kind="ExternalInput")
    yres = nc.dram_tensor("yres", [D, TC], F32, kind="ExternalInput")
    Wq8 = nc.dram_tensor("Wq8", [D, D], FP8, kind="ExternalInput")
    Wk8 = nc.dram_tensor("Wk8", [D, D], FP8, kind="ExternalInput")
    Wv8 = nc.dram_tensor("Wv8", [D, D], FP8, kind="ExternalInput")
    Wo8 = nc.dram_tensor("Wo8", [D, D], FP8, kind="ExternalInput")
    W18a = nc.dram_tensor("W18a", [D, DM], FP8, kind="ExternalInput")
    W18r = nc.dram_tensor("W18r", [D, DM], FP8, kind="ExternalInput")
    W28a = nc.dram_tensor("W28a", [DM, D], FP8, kind="ExternalInput")
    W28r = nc.dram_tensor("W28r", [DM, D], FP8, kind="ExternalInput")
    zrow = nc.dram_tensor("zrow", [1, 2048], FP8, kind="ExternalInput")
    bq8row = nc.dram_tensor("bq8row", [1, D], F32, kind="ExternalInput")
    nrows = nc.dram_tensor("nrows", [4, D], F32, kind="ExternalInput")
    b2row8 = nc.dram_tensor("b2row8", [1, D], F32, kind="ExternalInput")
    b18row = nc.dram_tensor("b18row", [1, DM], F32, kind="ExternalInput")
    ccol = nc.dram_tensor("ccol", [P, KC + MC1], F32, kind="ExternalInput")
    outT = nc.dram_tensor("outT", [D, TC], F32, kind="ExternalOutput")

    with tile.TileContext(nc, pool_alloc_mode="queue") as tc:
        with (
            tc.tile_pool(name="const", bufs=1) as const,
            tc.tile_pool(name="big", bufs=1) as big,
        ):
            # ---- persistent tiles ----
            x8 = big.tile([P, KC, N], FP8, name="x8")
            K8 = big.tile([P, KC, 2, N], FP8, name="K8")
            Q8 = big.tile([P, KC, TC], FP8, name="Q8")
            wq_sb = big.tile([P, KC, D], FP8, name="wq_sb")
            wk_sb = big.tile([P, KC, D], FP8, name="wk_sb")
            wv_sb = big.tile([P, KC, D], FP8, name="wv_sb")
            wo_sb = big.tile([P, KC, D], FP8, name="wo_sb")
            w1a_sb = big.tile([P, KC, DM], FP8, name="w1a_sb")
            w1r_sb = big.tile([P, KC, DM], FP8, name="w1r_sb")
            w2_sb = big.tile([P, MC1, D], FP8, name="w2_sb")
            w2r_sb = big.tile([P, MC1, D], FP8, name="w2r_sb")
            yr = big.tile([P, KC, TC], F32, name="yr")

            x8r = x8d.rearrange("(o p) t -> p o t", p=P)
            # SP queue: x tiles, K8 slot-1 zeros, MLP weights, residual
            for t in range(NT):
                ts = slice(t * 512, t * 512 + 512)
                nc.sync.dma_start(out=x8[:, :, ts], in_=x8r[:, :, ts])
            zsrc = bass.AP(tensor=zrow, offset=0,
                           ap=[[0, P], [0, KC], [1, 2048]])
            nc.sync.dma_start(out=K8[:, :, 1, :], in_=zsrc)
            nc.sync.dma_start(out=w1a_sb[:],
                              in_=W18a.rearrange("(o p) n -> p o n", p=P))
            nc.sync.dma_start(out=w1r_sb[:],
                              in_=W18r.rearrange("(o p) n -> p o n", p=P))
            nc.sync.dma_start(out=w2_sb[:],
                              in_=W28a.rearrange("(o p) n -> p o n", p=P))
            nc.sync.dma_start(out=w2r_sb[:],
                              in_=W28r.rearrange("(o p) n -> p o n", p=P))
            nc.sync.dma_start(out=yr[:],
                              in_=yres.rearrange("(o p) t -> p o t", p=P))
            # ACT queue: all four projection weights (ACT idle early)
            nc.scalar.dma_start(out=wk_sb[:],
                                in_=Wk8.rearrange("(o p) n -> p o n", p=P))
            nc.scalar.dma_start(out=wq_sb[:],
                                in_=Wq8.rearrange("(o p) n -> p o n", p=P))
            nc.scalar.dma_start(out=wv_sb[:],
                                in_=Wv8.rearrange("(o p) n -> p o n", p=P))
            nc.scalar.dma_start(out=wo_sb[:],
                                in_=Wo8.rearrange("(o p) n -> p o n", p=P))
            # Pool queue: rows/cols consts
            bq8_sb = const.tile([1, D], F32R, name="bq8_sb")
            nc.gpsimd.dma_start(out=bq8_sb[:], in_=bq8row[:])
            nrow_sb = const.tile([1, 4, D], F32R, name="nrow_sb")
            nc.gpsimd.dma_start(out=nrow_sb[:],
                                in_=nrows.rearrange("(o r) d -> o r d", o=1))
            b2r_sb = const.tile([1, D], F32R, name="b2r_sb")
            nc.gpsimd.dma_start(out=b2r_sb[:], in_=b2row8[:])
            b18_sb = const.tile([1, DM], F32R, name="b18_sb")
            nc.gpsimd.dma_start(out=b18_sb[:], in_=b18row[:])
            ccol_sb = const.tile([P, KC + MC1], F32, name="ccol_sb")
            nc.gpsimd.dma_start(out=ccol_sb[:], in_=ccol[:])

            # ---- constants ----
            ones8 = const.tile([P, 2, P], FP8, name="ones8")
            nc.vector.memset(ones8[:], 1.0 / D)      # 2^-9, exact in e4m3
            ident = const.tile([1, 1], F32)
            nc.vector.memset(ident[:], 1.0)
            eps1 = const.tile([1, 1], F32)
            nc.vector.memset(eps1[:], EPS)
            epsc = const.tile([P, 1], F32)
            nc.vector.memset(epsc[:], EPS)
            onec_f = const.tile([P, 1], F32)
            nc.vector.memset(onec_f[:], 1.0)
            inv8col = const.tile([P, 1], F32)
            nc.vector.memset(inv8col[:], 1.0 / S)
            onerow_f = const.tile([1, TC], F32)
            nc.vector.memset(onerow_f[:], 1.0)
            o8row_f = const.tile([1, P], F32)
            nc.vector.memset(o8row_f[:], 1.0 / S)
            odiv_f = const.tile([P, 1], F32)
            nc.vector.memset(odiv_f[:], 1.0 / D)
            # table pin while ACT is otherwise idle (sqrt first: LN1 cols)
            junk1 = const.tile([1, 1], F32)
            nc.scalar.activation(out=junk1[:], in_=eps1[:], func=AF.Sqrt)
            # f32r consts via Pool copies
            ones_row = const.tile([1, P], F32R, name="ones_row")
            nc.gpsimd.tensor_copy(out=ones_row[:], in_=onerow_f[:, 0:P])
            o8_row = const.tile([1, P], F32R, name="o8_row")
            nc.gpsimd.tensor_copy(out=o8_row[:], in_=o8row_f[:])
            ones64 = const.tile([1, HD], F32R, name="ones64")
            nc.gpsimd.tensor_copy(out=ones64[:], in_=onerow_f[:, 0:HD])
            ones512 = const.tile([1, TC], F32R, name="ones512")
            nc.gpsimd.tensor_copy(out=ones512[:], in_=onerow_f[:])
            od_col = const.tile([P, 1], F32R, name="od_col")
            nc.gpsimd.tensor_copy(out=od_col[:], in_=odiv_f[:])

            V8 = big.tile([P, JC, H, HD + 4], FP8, name="V8")
            st8a = big.tile([P, JC, 2, TC], FP8, name="st8a")
            st8b = big.tile([P, JC, 2, TC], FP8, name="st8b")
            RT = big.tile([P, KC, TC], FP8, name="RT")
            y2T = big.tile([P, KC, TC], F32R, name="y2T")
            arepq = big.tile([P, TC], F32, name="arepq")

            nc.vector.memset(V8[:, :, :, HD + 1:HD + 4], 0.0)
            nc.vector.tensor_copy(out=V8[:, :, :, HD:HD + 1],
                                  in_=onec_f.to_broadcast((P, JC, H, 1)))

            rows = ExitStack()
            rowsA = rows.enter_context(tc.tile_pool(name="rowsA", bufs=1))
            sqp = rows.enter_context(tc.tile_pool(name="sqp", bufs=2))
            rpool = rows.enter_context(tc.tile_pool(name="rpool", bufs=2))
            sd_row = rowsA.tile([1, 512], F32R, name="sd_row")
            rstd_row = rowsA.tile([1, 512], F32R, name="rstd_row")
            sd_col = rowsA.tile([P, JC], F32, name="sd_col")
            rstd_col = rowsA.tile([P, JC], F32, name="rstd_col")
            s64_col = rowsA.tile([P, JC], F32, name="s64_col")
            s64A_col = rowsA.tile([P, JC], F32, name="s64A_col")
            rs8_col = rowsA.tile([P, JC], F32, name="rs8_col")
            vrow = [rowsA.tile([1, 512], F32, name=f"vrow{t}")
                    for t in range(NT)]
            mrow_t = [rowsA.tile([1, 512], F32R, name=f"mrow{t}")
                      for t in range(NT)]

            gemm_stack = ExitStack()
            gemm = gemm_stack.enter_context(
                tc.tile_pool(name="gemm", bufs=2, space="PSUM"))
            stat_stack = ExitStack()
            stat = stat_stack.enter_context(
                tc.tile_pool(name="stat", bufs=1, space="PSUM"))

            # ---- LN1 stats: var from raw x8 (independent of xc) ----
            for t in range(NT):
                ts = slice(t * 512, t * 512 + 512)
                pm = stat.tile([P, 512], F32, name=f"pm{t}", tag="pm")
                for a in range(2):
                    nc.tensor.matmul(pm[:], ones8[:],
                                     x8[:, 2 * a:2 * a + 2, ts],
                                     start=(a == 0), stop=(a == 1),
                                     perf_mode=DR, skip_group_check=True)
                sq = sqp.tile([P, KC, 512], FP8, name=f"sq{t}", tag="sq")
                sq_eng = nc.gpsimd if t % 2 == 0 else nc.vector
                sq_eng.tensor_tensor(out=sq[:], in0=x8[:, :, ts],
                                     in1=x8[:, :, ts], op=OP.mult)
                mrow = mrow_t[t]
                # mean-row copy on ACT (Copy lives in every act table)
                nc.scalar.activation(out=mrow.bitcast(F32)[:],
                                     in_=pm[0:1, :], func=AF.Copy)
                ps = stat.tile([P, 512], F32, name=f"ps{t}", tag="ps")
                for a in range(2):
                    nc.tensor.matmul(ps[:], ones8[:],
                                     sq[:, 2 * a:2 * a + 2, :],
                                     start=(a == 0), stop=(a == 1),
                                     perf_mode=DR, skip_group_check=True)
                nc.gpsimd.tensor_copy(out=vrow[t][:], in_=ps[0:1, :])
                # transposes land in the (now dead) tail of the ps tile
                for i in range(4):
                    nc.tensor.transpose(ps[:, 496 + i:497 + i],
                                        vrow[t][:, i * P:(i + 1) * P],
                                        ident[:])
                    nc.tensor.transpose(ps[:, 488 + i:489 + i],
                                        mrow.bitcast(F32)[:,
                                                         i * P:(i + 1) * P],
                                        ident[:])
                cs = slice(4 * t, 4 * t + 4)
                msq_c = rowsA.tile([P, 4], F32, name=f"msqc{t}")
                nc.vector.tensor_tensor(out=msq_c[:], in0=ps[:, 488:492],
                                        in1=ps[:, 488:492], op=OP.mult)
                var_c = rowsA.tile([P, 4], F32, name=f"varc{t}")
                nc.vector.tensor_tensor(out=var_c[:], in0=ps[:, 496:500],
                                        in1=msq_c[:], op=OP.subtract)
                nc.scalar.activation(out=sd_col[:, cs], in_=var_c[:],
                                     func=AF.Sqrt, bias=epsc[:])
                nc.vector.reciprocal(out=rstd_col[:, cs], in_=sd_col[:, cs])
                nc.vector.tensor_scalar(out=s64_col[:, cs],
                                        in0=rstd_col[:, cs],
                                        scalar1=1.0 / (S * SQH),
                                        scalar2=None, op0=OP.mult)
                nc.vector.tensor_scalar(out=s64A_col[:, cs],
                                        in0=rstd_col[:, cs],
                                        scalar1=A8 / (S * SQH),
                                        scalar2=None, op0=OP.mult)
                nc.vector.tensor_scalar(out=rs8_col[:, cs],
                                        in0=rstd_col[:, cs],
                                        scalar1=1.0 / (S * S),
                                        scalar2=None, op0=OP.mult)
                if t == 0:
                    # row-space sd/rstd for the Q scale + bias fold
                    msq_r = rowsA.tile([1, 512], F32, name="msqr0")
                    nc.vector.tensor_tensor(out=msq_r[:],
                                            in0=mrow.bitcast(F32)[:],
                                            in1=mrow.bitcast(F32)[:],
                                            op=OP.mult)
                    var_r = rowsA.tile([1, 512], F32, name="varr0")
                    nc.vector.tensor_tensor(out=var_r[:], in0=vrow[0][:],
                                            in1=msq_r[:], op=OP.subtract)
                    nc.scalar.activation(out=sd_row.bitcast(F32)[:],
                                         in_=var_r[:], func=AF.Sqrt,
                                         bias=eps1[:])
                    with nc.allow_low_precision(reason="f32r == f32 bits"):
                        nc.vector.reciprocal(out=rstd_row[:],
                                             in_=sd_row.bitcast(F32)[:])
                    aq = gemm.tile([P, 512], F32, name="aq", tag="pk")
                    nc.tensor.matmul(aq[:], o8_row[:], rstd_row[:],
                                     start=True, stop=True,
                                     skip_group_check=True)
                    nc.gpsimd.tensor_copy(out=arepq[:], in_=aq[:])

            # ACT exp scale rides a copy made after t3 stats, so every ACT
            # exp is scheduled after all four LN1 col sqrts (no act-table
            # ping-pong); D/P schraudolph units use s64A_col directly.
            s64w_col = rowsA.tile([P, JC], F32, name="s64w_col")
            nc.vector.tensor_copy(out=s64w_col[:], in_=s64_col[:])
            s64Aw_col = rowsA.tile([P, JC], F32, name="s64Aw_col")
            nc.vector.tensor_copy(out=s64Aw_col[:], in_=s64A_col[:])
            # exp table pin after the last LN1 col sqrt
            nc.scalar.activation(out=junk1[:], in_=sd_col[0:1, 15:16],
                                 func=AF.Exp)

            NSQ, NSK, NSV = 0, 1, 2

            def k_nt(m, t):
                ms = slice(m * P, m * P + P)
                ts = slice(t * 512, t * 512 + 512)
                pk = gemm.tile([P, 512], F32, name="pk", tag="pk")
                nc.tensor.matmul(pk[:], nrow_sb[:, NSK, ms], mrow_t[t][:],
                                 start=True, stop=False,
                                 skip_group_check=True)
                for a in range(2):
                    nc.tensor.matmul(pk[:], wk_sb[:, 2 * a:2 * a + 2, ms],
                                     x8[:, 2 * a:2 * a + 2, ts],
                                     start=False, stop=(a == 1),
                                     perf_mode=DR, skip_group_check=True)
                nc.gpsimd.tensor_copy(out=K8[:, m, 0, ts], in_=pk[:])

            def q_m(m):
                ms = slice(m * P, m * P + P)
                pq = gemm.tile([P, 512], F32, name="pq", tag="pk")
                nc.tensor.matmul(pq[:], bq8_sb[:, ms], sd_row[:],
                                 start=True, stop=False,
                                 skip_group_check=True)
                nc.tensor.matmul(pq[:], nrow_sb[:, NSQ, ms], mrow_t[0][:],
                                 start=False, stop=False,
                                 skip_group_check=True)
                for a in range(2):
                    nc.tensor.matmul(pq[:], wq_sb[:, 2 * a:2 * a + 2, ms],
                                     x8[:, 2 * a:2 * a + 2, 0:TC],
                                     start=False, stop=(a == 1),
                                     perf_mode=DR, skip_group_check=True)
                nc.vector.tensor_tensor(out=Q8[:, m, :], in0=pq[:],
                                        in1=arepq[:], op=OP.mult)

            def v_chunk(jc):
                js = slice(jc * P, jc * P + P)
                t = jc // 4
                jl = slice((jc % 4) * P, (jc % 4) * P + P)
                pv = gemm.tile([P, 512], F32, name="pv", tag="pk")
                nc.tensor.matmul(pv[:], mrow_t[t][:, jl], nrow_sb[:, NSV, :],
                                 start=True, stop=False,
                                 skip_group_check=True)
                for a in range(2):
                    nc.tensor.matmul(pv[:], x8[:, 2 * a:2 * a + 2, js],
                                     wv_sb[:, 2 * a:2 * a + 2, :],
                                     start=False, stop=(a == 1),
                                     perf_mode=DR, skip_group_check=True)
                eng = nc.vector if jc % 2 == 0 else nc.gpsimd
                eng.tensor_scalar(
                    out=V8[:, jc, :, 0:HD],
                    in0=pv.rearrange("p (h c) -> p h c", h=H),
                    scalar1=rs8_col[:, jc:jc + 1], scalar2=None, op0=OP.mult)

            def av_head(m, r):
                """A@V for head 2m+r reading st8 buffer of wave m.
                pr comes from the retired stats rings (pm for r=0, ps r=1)."""
                st8 = st8a if m % 2 == 0 else st8b
                h = 2 * m + r
                hs = slice(r * HD, r * HD + HD)
                pr = stat.tile([HD + 4, 512], F32, name="pr",
                               tag=("pm" if r == 0 else "ps"))
                for j in range(JC // 2):
                    nc.tensor.matmul(
                        pr[:], V8[:, 2 * j:2 * j + 2, h, :],
                        st8[:, 2 * j:2 * j + 2, r, :],
                        start=(j == 0), stop=(j == JC // 2 - 1),
                        perf_mode=DR)
                rs_row = rpool.tile([1, TC], F32R, name="rs", tag="rs")
                with nc.allow_low_precision(reason="f32r == f32 bits"):
                    nc.vector.reciprocal(out=rs_row[:],
                                         in_=pr[HD:HD + 1, :])
                rrep_p = gemm.tile([P, 512], F32, name="rrp", tag="pk")
                nc.tensor.matmul(rrep_p[0:HD, :], ones64[:], rs_row[:],
                                 start=True, stop=True,
                                 skip_group_check=True)
                rrep = rpool.tile([HD, TC], F32, name="rrep", tag="rrep")
                nc.gpsimd.tensor_copy(out=rrep[:], in_=rrep_p[0:HD, :])
                nc.gpsimd.tensor_tensor(out=RT[hs, m, :], in0=pr[0:HD, :],
                                          in1=rrep[:], op=OP.mult)

            q_m(0)
            k_nt(0, 0)
            k_nt(0, 1)

            pss_stack = ExitStack()
            pss = pss_stack.enter_context(
                tc.tile_pool(name="pss", bufs=4, space="PSUM"))

            # per-m insert work lists, one item per (jc, r) slot
            inserts = {
                0: [lambda: k_nt(0, 2), lambda: k_nt(0, 3), lambda: q_m(1)]
                   + [(lambda jc=jc: v_chunk(jc)) for jc in range(6)]
                   + [(lambda t=t: k_nt(1, t)) for t in range(NT)]
                   + [(lambda jc=jc: v_chunk(jc)) for jc in (6, 7)],
                1: [lambda: q_m(2)]
                   + [(lambda jc=jc: v_chunk(jc)) for jc in range(8, JC)]
                   + [(lambda t=t: k_nt(2, t)) for t in range(NT)]
                   + [lambda: av_head(0, 0), lambda: av_head(0, 1)],
                2: [lambda: q_m(3)]
                   + [(lambda t=t: k_nt(3, t)) for t in range(NT)]
                   + [lambda: av_head(1, 0), lambda: av_head(1, 1)],
                3: [lambda: av_head(2, 0), lambda: av_head(2, 1)],
            }

            st8u_a = st8a.bitcast(U8)
            st8u_b = st8b.bitcast(U8)
            last_act_unit = [None]

            def unit(m, jc, r, idx):
                st8 = st8a if m % 2 == 0 else st8b
                st8u = st8u_a if m % 2 == 0 else st8u_b
                js = slice(jc * P, jc * P + P)
                rsl = slice(r * HD, r * HD + HD)
                psc = pss.tile([P, 512], F32, name="psc", tag="psc")
                nc.tensor.matmul(
                    psc[:], K8[rsl, m, :, js],
                    mid_bcast(Q8[rsl, m, :], 2),
                    start=True, stop=True, perf_mode=DR,
                    skip_group_check=True)
                w = EXP_SCHED[idx]
                if w == "A":
                    nc.scalar.activation(
                        out=st8[:, jc, r, :], in_=psc[:],
                        func=AF.Exp, scale=s64w_col[:, jc:jc + 1])
                    last_act_unit[0] = (st8, jc, r)
                else:
                    eng = nc.vector if w == "D" else nc.gpsimd
                    eng.tensor_scalar(
                        out=st8u[:, jc, r, :], in0=psc[:],
                        scalar1=s64Aw_col[:, jc:jc + 1], scalar2=B8,
                        op0=OP.mult, op1=OP.add)

            for m in range(3):
                todo = list(inserts[m])
                for jc in range(JC):
                    for r in range(2):
                        unit(m, jc, r, (m * JC + jc) * 2 + r)
                        if todo and (m > 0 or jc >= 1):
                            todo.pop(0)()
                for fn in todo:
                    fn()
            # wave 3 runs r-major so av_head(3,0) overlaps the r=1 stream
            todo = list(inserts[3])
            for r in range(2):
                for jc in range(JC):
                    unit(3, jc, r, 96 + r * JC + jc)
                    if todo:
                        todo.pop(0)()
                if r == 0:
                    av_head(3, 0)
            av_head(3, 1)
            # sqrt table prefetch for LN2: anchored on the last ACT exp so
            # the scheduler cannot hoist it into the exp stream
            st8l, jcl, rl = last_act_unit[0]
            nc.scalar.activation(out=junk1[:],
                                 in_=st8l[0:1, jcl, rl, 0:1], func=AF.Sqrt)

            pss_stack.close()
            stat_stack.close()
            gemm_stack.close()
            rows.close()

            # ================= phase C =================
            with (
                tc.tile_pool(name="phc", bufs=1) as phc,
                tc.tile_pool(name="rowsC", bufs=1) as rowsC,
                tc.tile_pool(name="pmmC", bufs=2, space="PSUM") as pmmC,
            ):
                pstatC_stack = ExitStack()
                pstatC = pstatC_stack.enter_context(
                    tc.tile_pool(name="pstatC", bufs=1, space="PSUM"))
                y2f = y2T.bitcast(F32)
                m2row = rowsC.tile([1, TC], F32R, name="m2row")
                msq2_row = rowsC.tile([1, TC], F32, name="msq2")
                var2_row = rowsC.tile([1, TC], F32, name="var2")
                sd2_row = rowsC.tile([1, TC], F32, name="sd2")
                rstd2_row = rowsC.tile([1, TC], F32R, name="rstd2")
                pm2 = pstatC.tile([1, 512], F32, name="pm2", tag="pm2")
                ps2 = pstatC.tile([P, 512], F32, name="ps2", tag="ps2")
                sq2 = phc.tile([P, KC, TC], FP8, name="sq2")
                for m in range(KC):
                    ms = slice(m * P, m * P + P)
                    po = pmmC.tile([P, 512], F32, name="po", tag="po")
                    for a in range(2):
                        nc.tensor.matmul(po[:], wo_sb[:, 2 * a:2 * a + 2, ms],
                                         RT[:, 2 * a:2 * a + 2, :],
                                         start=(a == 0), stop=(a == 1),
                                         perf_mode=DR)
                    eng = nc.vector if m % 2 == 0 else nc.gpsimd
                    eng.scalar_tensor_tensor(
                        out=y2T[:, m, :], in0=po[:],
                        scalar=ccol_sb[:, m:m + 1],
                        in1=yr[:, m, :], op0=OP.add, op1=OP.add)
                    nc.tensor.matmul(pm2[:], od_col[:], y2T[:, m, :],
                                     start=(m == 0), stop=(m == KC - 1))
                    eng2 = nc.gpsimd if m % 2 == 0 else nc.vector
                    eng2.tensor_tensor(out=sq2[:, m, :], in0=y2f[:, m, :],
                                       in1=y2f[:, m, :], op=OP.mult)
                for a in range(2):
                    nc.tensor.matmul(ps2[:], ones8[:],
                                     sq2[:, 2 * a:2 * a + 2, :],
                                     start=(a == 0), stop=(a == 1),
                                     perf_mode=DR, skip_group_check=True)
                with nc.allow_low_precision(reason="f32r == f32 bits"):
                    nc.gpsimd.tensor_copy(out=m2row[:], in_=pm2[:])
                nc.vector.tensor_tensor(out=msq2_row[:],
                                        in0=m2row.bitcast(F32)[:],
                                        in1=m2row.bitcast(F32)[:],
                                        op=OP.mult)
                nc.vector.tensor_tensor(out=var2_row[:], in0=ps2[0:1, :],
                                        in1=msq2_row[:], op=OP.subtract)
                nc.scalar.activation(out=sd2_row[:], in_=var2_row[:],
                                     func=AF.Sqrt, bias=eps1[:])
                # gelu table prefetch right behind the LN2 sqrt
                nc.scalar.activation(out=junk1[:], in_=sd2_row[0:1, 0:1],
                                     func=AF.Gelu)
                with nc.allow_low_precision(reason="f32r == f32 bits"):
                    nc.vector.reciprocal(out=rstd2_row[:], in_=sd2_row[:])
                mrep2 = pstatC.tile([P, 512], F32, name="mrep2", tag="mrep2")
                nc.tensor.matmul(mrep2[:], ones_row[:], m2row[:],
                                 start=True, stop=True)
                rrep2 = pstatC.tile([P, 512], F32, name="rrep2", tag="rrep2")
                nc.tensor.matmul(rrep2[:], ones_row[:], rstd2_row[:],
                                 start=True, stop=True)
                zt = phc.tile([P, KC, TC], F32, name="zt")
                z8 = phc.tile([P, KC, TC], FP8, name="z8")
                for k in range(KC):
                    eng = nc.vector if k % 2 == 0 else nc.gpsimd
                    eng.tensor_tensor(out=zt[:, k, :], in0=y2f[:, k, :],
                                      in1=mrep2[:], op=OP.subtract)
                for k in range(KC):
                    eng = nc.vector if k % 2 == 0 else nc.gpsimd
                    eng.tensor_tensor(out=z8[:, k, :], in0=zt[:, k, :],
                                      in1=rrep2[:], op=OP.mult)

                pstatC_stack.close()
                # MLP: h2 accumulates on DoubleRow pairs as h1 chunks land
                h1 = phc.tile([P, MC1, TC], FP8, name="h1")
                ph2_stack = ExitStack()
                ph2 = ph2_stack.enter_context(
                    tc.tile_pool(name="ph2", bufs=1, space="PSUM"))
                p2s = [ph2.tile([P, 512], F32, name=f"p2_{m}", tag=f"p2_{m}")
                       for m in range(KC)]
                out_sb = phc.tile([P, KC, TC], F32, name="out_sb")
                outT_r = outT.rearrange("(o p) t -> p o t", p=P)
                for m in range(KC):
                    ms = slice(m * P, m * P + P)
                    nc.tensor.matmul(p2s[m][:], b2r_sb[:, ms], ones512[:],
                                     start=True, stop=False,
                                     skip_group_check=True)
                for kp in range(MC1 // 2):
                    p1 = pmmC.tile([P, 2, 512], F32, name="p1", tag="po")
                    for half in range(2):
                        k_ = 2 * kp + half
                        ks = slice(k_ * P, k_ * P + P)
                        nc.tensor.matmul(p1[:, half, :], b18_sb[:, ks],
                                         ones512[:], start=True, stop=False,
                                         skip_group_check=True)
                        for a in range(2):
                            nc.tensor.matmul(
                                p1[:, half, :],
                                w1a_sb[:, 2 * a:2 * a + 2, ks],
                                z8[:, 2 * a:2 * a + 2, :],
                                start=False, stop=False,
                                perf_mode=DR, skip_group_check=True)
                        for a in range(2):
                            nc.tensor.matmul(
                                p1[:, half, :],
                                w1r_sb[:, 2 * a:2 * a + 2, ks],
                                z8[:, 2 * a:2 * a + 2, :],
                                start=False, stop=(a == 1),
                                perf_mode=DR, skip_group_check=True)
                    k = 2 * kp + 1
                    nc.scalar.activation(
                        out=h1[:, k - 1:k + 1, :], in_=p1[:],
                        func=AF.Gelu, scale=1.0 / S)
                    if True:
                        for m in range(KC):
                            ms = slice(m * P, m * P + P)
                            nc.tensor.matmul(
                                p2s[m][:], w2_sb[:, k - 1:k + 1, ms],
                                h1[:, k - 1:k + 1, :],
                                start=False, stop=False,
                                perf_mode=DR, skip_group_check=True)
                            nc.tensor.matmul(
                                p2s[m][:], w2r_sb[:, k - 1:k + 1, ms],
                                h1[:, k - 1:k + 1, :],
                                start=False, stop=(k == MC1 - 1),
                                perf_mode=DR, skip_group_check=True)
                        if k == MC1 - 1:
                            for m in range(KC):
                                eng = nc.vector if m % 2 == 0 else nc.gpsimd
                                eng.scalar_tensor_tensor(
                                    out=out_sb[:, m, :], in0=p2s[m][:],
                                    scalar=inv8col[:],
                                    in1=y2f[:, m, :],
                                    op0=OP.mult, op1=OP.add)
                                dma_eng = (nc.sync, nc.scalar, nc.gpsimd,
                                           nc.sync)[m]
                                dma_eng.dma_start(out=outT_r[:, m, :],
                                                  in_=out_sb[:, m, :])
                ph2_stack.close()

    return _finalize(nc) if do_finalize else nc


def prep_inputs(y, Wq, bq, Wk, bk, Wv, bv, Wo, bo, ln1_g, ln1_b, ln2_g, ln2_b,
                W1, b1, W2, b2):
    """Host-side weight folding + per-core input maps."""
    f = np.float32
    E4 = mybir.dt.np(FP8)
    Wq_ = (Wq * ln1_g[:, None]).astype(f)
    Wk_ = (Wk * ln1_g[:, None]).astype(f)
    Wv_ = (Wv * ln1_g[:, None]).astype(f)
    bq_ = (ln1_b @ Wq + bq).astype(f)
    bv_ = (ln1_b @ Wv + bv).astype(f)
    bo_ = (bv_ @ Wo + bo).astype(f)
    W1_ = (W1 * ln2_g[:, None]).astype(f)
    b1_ = (ln2_b @ W1 + b1).astype(f)

    bq8row = (S * bq_).reshape(1, D).astype(f)
    nrows = np.stack([S * (-Wq_.sum(0)), S * (-Wk_.sum(0)),
                      S * (-Wv_.sum(0)), np.zeros(D, f)]).astype(f)
    b2row8 = (S * np.asarray(b2, f)).reshape(1, D).astype(f)
    b18row = (S * b1_).reshape(1, DM).astype(f)
    ccol = np.concatenate([
        bo_.reshape(KC, P).T, b1_.reshape(MC1, P).T,
    ], axis=1).astype(f)

    W18a = (S * W1_).astype(E4)
    W18r = (S * W1_ - W18a.astype(f)).astype(E4)
    W28a = (S * np.asarray(W2, f)).astype(E4)
    W28r = (S * np.asarray(W2, f) - W28a.astype(f)).astype(E4)
    shared = {
        "Wq8": np.ascontiguousarray(S * Wq_).astype(E4),
        "Wk8": np.ascontiguousarray(S * Wk_).astype(E4),
        "Wv8": np.ascontiguousarray(S * Wv_).astype(E4),
        "Wo8": np.ascontiguousarray(S * np.asarray(Wo, f)).astype(E4),
        "W18a": np.ascontiguousarray(W18a),
        "W18r": np.ascontiguousarray(W18r),
        "W28a": np.ascontiguousarray(W28a),
        "W28r": np.ascontiguousarray(W28r),
        "zrow": np.zeros((1, 2048), mybir.dt.np(FP8)),
        "nrows": nrows, "b18row": b18row,
        "bq8row": bq8row, "b2row8": b2row8, "ccol": ccol,
    }
    in_maps = []
    for c in range(8):
        b, s_ = divmod(c, 4)
        ts = s_ * TC
        yTm = np.asarray(y, np.float32)[b].T
        yrot = np.ascontiguousarray(np.roll(yTm, -ts, axis=1))
        in_maps.append({
            "x8d": yrot.astype(E4),
            "yres": np.ascontiguousarray(yrot[:, 0:TC]),
            **shared,
        })
    return in_maps


def gather_output(results):
    out = np.empty((B, N, D), np.float32)
    for c in range(8):
        b, s_ = divmod(c, 4)
        out[b, s_ * TC:(s_ + 1) * TC, :] = results[c]["outT"].T
    return out


_NC_CACHE = {}


def kernel(**inputs):
    """Full-input entry point: shard, run on 8 NeuronCores, gather."""
    from concourse.bass_utils import run_bass_kernel_spmd

    in_maps = prep_inputs(**{k: np.asarray(v) for k, v in inputs.items()})
    if "nc" not in _NC_CACHE:
        _NC_CACHE["nc"] = build_nc()
    nc = _NC_CACHE["nc"]
    res = run_bass_kernel_spmd(nc, in_maps, core_ids=list(range(8)))
    return gather_output(res.results)
